# revision 30
# baseline (speedup 1.0000x reference)
"""ChamferLoss2D Trainium2 kernel (8 NeuronCores, SPMD).

Problem: three point sets [4, 4096, 2]; pairwise chamfer losses between
(p1,p2), (p1,p3), (p2,p3); output[b] = MARGIN - mean of the three
chamfer distances.

Algorithm:
  - For each (pair, batch): squared-distance matrix sq[n, m] computed
    entirely on the TensorEngine as a K=18 bf16 matmul using 3-way
    hi/mid/lo bf16 splits of (-2x), y, |x|^2, |y|^2 (error ~2e-7, on
    par with fp32 reference numerics).
  - min(sqrt(sq)) = sqrt(min(sq)): only row/col mins of sq are needed
    on device; sqrt + means happen on host.
  - ScalarE casts PSUM fp32 -> SBUF bf16; VectorE does both min passes
    in bf16 at 2x (tensor_tensor min; row direction via a fold chain
    batched over 4 x-tiles with 3D APs, then one small tensor_reduce).
  - Sharding: 12 (pair, batch) units x 2 halves of the x/query axis =
    24 half-units, 3 per core. Row mins come back fully reduced; col
    mins come back as [128, 4096] partials, min-combined on host.
"""

import numpy as np
import ml_dtypes

BF16 = ml_dtypes.bfloat16

B = 4
N = 4096
D = 2
MARGIN = 1.0
LOSS_WEIGHT = 1.0

N_CORES = 8
HALF = N // 2           # 2048 x-points per half-unit
XT = HALF // 128        # 16 x-tiles per half-unit
UNITS_PER_CORE = 3
K = 18                  # matmul contraction rows

PAIRS = ((0, 1), (0, 2), (1, 2))
# 24 half-units: (pair_idx, batch, half) in fixed order, 3 per core.
UNITS = [(p, b, h) for p in range(3) for b in range(B) for h in range(2)]

_NC_CACHE = {}


def _split3(v64):
    """3-way bf16 split of a float64 array: v ~= h + m + l (residual ~2^-27)."""
    h = v64.astype(BF16)
    r = v64 - h.astype(np.float64)
    m = r.astype(BF16)
    r2 = r - m.astype(np.float64)
    l = r2.astype(BF16)
    return h, m, l


# Engine-completion sems are named "<proc>_<n>". An instruction waiting on
# its OWN engine's completion sem is redundant: all five engines complete
# in program order (PE MMs end pc-monotone; DVE/ACT drain per op), so by
# issue time every earlier own-engine instruction has already bumped the
# sem. DMA-queue sems (DMASW*/DMAHW*) are NOT engine-ordered - keep those.
_ENGINE_SEM_PREFIX = {
    "PE": "PE_",
    "Activation": "Activation_",
    "DVE": "DVE_",
    "Pool": "Pool_",
    "SP": "SP_",
}


def _legalize_sync_waits(nc, sem_by_name):
    """This image's walrus rejects >1 sem-wait on many instruction structs.

    1. Drop redundant own-engine completion waits.
    2. Keep the first remaining wait on the instruction; hoist extras onto
       wait_ge (InstEventSemaphore) carriers inserted immediately before it
       on the same engine (per-engine program order is list order within a
       basic block). Carriers are emitted via the real engine builders (so
       they are well-formed), then relocated."""

    def grab_carrier(engine, sem, value):
        bi = nc.engines[engine].wait_ge(sem, value)
        carrier = bi.ins
        # The builder appended it to the current (tail) bb; remove it.
        cur = nc.cur_bb.bb
        tl = cur.instructions
        assert tl[-1].name == carrier.name, (tl[-1].name, carrier.name)
        cur.instructions = tl[:-1]
        return carrier

    for f in nc.m.functions:
        for bb in f.blocks:
            insts = list(bb.instructions)
            out = []
            changed = False
            for inst in insts:
                si = inst.sync_info
                waits = list(si.on_wait) if si is not None else []
                if len(waits) > 1:
                    pfx = _ENGINE_SEM_PREFIX.get(getattr(inst.engine, "value", ""))
                    if pfx is not None:
                        kept = [w for w in waits if not w.ant_name.startswith(pfx)]
                    else:
                        kept = waits
                    for w in kept[1:]:
                        h = sem_by_name.get(w.ant_name)
                        if h is None:
                            raise RuntimeError(f"unknown sem {w.ant_name}")
                        out.append(grab_carrier(inst.engine, h, w.wait_value))
                    si.on_wait = kept[:1]
                    inst.sync_info = si
                    changed = True
                out.append(inst)
            if changed:
                bb.instructions = out


def _make_patched_tile_context():
    """Tail-drain workaround + global sync-wait legalization."""
    from concourse import tile
    from concourse.vector_clock import ScopedClock

    class PatchedTileContext(tile.TileContext):
        def _drain_and_barrier(self, tick_clock, wait_clock):
            nc = self.nc
            assert self.sems is not None
            sem_by_name = {h.name: h for h in self.sems.allocated().values()}
            _legalize_sync_waits(nc, sem_by_name)
            carrier = nc.sync.nop()
            wait_clock.add_sem_waits(
                carrier.ins, ScopedClock({None: tick_clock.global_clock})
            )
            waits = list(carrier.ins.sync_info.on_wait)
            if waits:
                si = carrier.ins.sync_info
                si.on_wait = []
                carrier.ins.sync_info = si
                for w in waits:
                    h = sem_by_name.get(w.ant_name)
                    if h is None:
                        raise RuntimeError(f"unknown tail sem {w.ant_name}")
                    nc.sync.wait_ge(h, w.wait_value)
            nc.sync.drain()

            # Minimal tail: the SP waits above already gate on all engine /
            # DMA completion sems; skip the expensive EVSEM butterfly
            # (2x all-engine barrier + 27 sem clears, ~10us) that the stock
            # TileContext emits. Each engine's stream simply ends; NEFF
            # completion waits for all engines and DMA queues regardless.
            popped = nc._tile_sem_poison_stack.pop()
            assert popped is self._sem_poison

    return PatchedTileContext


def _build_nc():
    import concourse.bass as bass
    from concourse import mybir

    PatchedTileContext = _make_patched_tile_context()
    dt = mybir.dt
    AluOp = mybir.AluOpType

    nc = bass.Bass(trn_type="TRN2")
    lhst_in = nc.dram_tensor(
        "lhst_in", [UNITS_PER_CORE, K, HALF], dt.bfloat16, kind="ExternalInput"
    )
    rhs_in = nc.dram_tensor(
        "rhs_in", [UNITS_PER_CORE, K, N], dt.bfloat16, kind="ExternalInput"
    )
    # col-direction: ship per-pair-of-x-tiles partials; the remaining min
    # folds run on the host (DVE relief; DMA engines are nearly idle)
    colmin_out = nc.dram_tensor(
        "colmin_out",
        [UNITS_PER_CORE, XT // 4, 128, 2 * N],
        dt.bfloat16,
        kind="ExternalOutput",
    )
    rowmin_out = nc.dram_tensor(
        "rowmin_out", [UNITS_PER_CORE, 128, XT], dt.float32, kind="ExternalOutput"
    )

    with PatchedTileContext(nc) as tc:
        with (
            tc.tile_pool(name="weights", bufs=2) as wpool,
            tc.tile_pool(name="sq", bufs=3) as sqpool,
            tc.tile_pool(name="acc", bufs=2) as accpool,
            tc.tile_pool(name="tmp", bufs=2) as tmppool,
            tc.tile_pool(name="psum", bufs=2, space="PSUM") as pspool,
        ):
            for u in range(UNITS_PER_CORE):
                # operands replicated at partition offsets 0/32/64/96 so four
                # K=18 matmuls can run concurrently in distinct PE row groups
                lhsT = wpool.tile([96 + K, HALF], dt.bfloat16, tag="lhsT")
                rhs = wpool.tile([96 + K, N], dt.bfloat16, tag="rhs")
                for m in range(4):
                    nc.sync.dma_start(lhsT[32 * m : 32 * m + K, :], lhst_in[u])
                    nc.sync.dma_start(rhs[32 * m : 32 * m + K, :], rhs_in[u])

                rowmins = accpool.tile([128, XT], dt.float32, tag="rowmins")

                for g in range(XT // 4):  # groups of 4 x-tiles
                    # chunk-major layout: sq4[p, chunk j, member m, 512]
                    # so each chunk's cast writes one contiguous 2048 span
                    sq4 = sqpool.tile([128, 8, 4, 512], dt.bfloat16, tag="sq4")
                    for j in range(8):  # 512-wide m-chunks
                        ps = pspool.tile([128, 2048], dt.float32, tag="ps")
                        for m in range(4):  # concurrent row-group members
                            i = 4 * g + m
                            nc.tensor.matmul(
                                ps[:, 512 * m : 512 * (m + 1)],
                                lhsT[32 * m : 32 * m + K, 128 * i : 128 * (i + 1)],
                                rhs[32 * m : 32 * m + K, 512 * j : 512 * (j + 1)],
                                tile_position=(32 * m, 0),
                            )
                        # fp32 PSUM -> bf16 SBUF cast on ScalarE (contiguous)
                        nc.scalar.copy(
                            sq4[:, j, :, :],
                            ps[:].rearrange("p (m f) -> p m f", m=4),
                        )
                    # col-direction: one batched pair-min (members {0,2} vs
                    # {1,3}) at DVE 2x, then DMA the partial to DRAM
                    pc = tmppool.tile([128, 8, 2, 512], dt.bfloat16, tag="pc")
                    nc.vector.tensor_tensor(
                        pc[:, :, :, :],
                        sq4[:, :, 0:4:2, :],
                        sq4[:, :, 1:4:2, :],
                        op=AluOp.min,
                    )
                    nc.sync.dma_start(colmin_out[u, g], pc[:])
                    # row-direction: fold chunks 8->4->2->1, then within-512,
                    # batched across the 4 members
                    rt = tmppool.tile([128, 4, 4, 512], dt.bfloat16, tag="rt")
                    nc.vector.tensor_tensor(
                        rt[:, :, :, :],
                        sq4[:, 0:4, :, :],
                        sq4[:, 4:8, :, :],
                        op=AluOp.min,
                    )
                    for clev in (2, 1):
                        nc.vector.tensor_tensor(
                            rt[:, :clev, :, :],
                            rt[:, :clev, :, :],
                            rt[:, clev : 2 * clev, :, :],
                            op=AluOp.min,
                        )
                    for wlev in (256, 128, 64):
                        nc.vector.tensor_tensor(
                            rt[:, 0, :, :wlev],
                            rt[:, 0, :, :wlev],
                            rt[:, 0, :, wlev : 2 * wlev],
                            op=AluOp.min,
                        )
                    nc.vector.tensor_reduce(
                        rowmins[:, 4 * g : 4 * g + 4],
                        rt[:, 0, :, :64],
                        axis=mybir.AxisListType.X,
                        op=AluOp.min,
                    )

                nc.sync.dma_start(rowmin_out[u], rowmins[:])

    return nc


def _get_nc():
    if "nc" not in _NC_CACHE:
        _NC_CACHE["nc"] = _build_nc()
    return _NC_CACHE["nc"]


def _prep_unit(x64, y64):
    """Build lhsT [K, n_x] and rhs [K, N] bf16 planes for one half-unit.

    sq[n, m] = |x_n|^2 + |y_m|^2 - 2 x_n . y_m, via 3-way bf16 splits:
    per dim d: a = -2 x_d, kept products (ah,yh),(ah,ym),(ah,yl),
    (am,yh),(am,ym),(al,yh); plus (vh|vm|vl, 1) and (1, wh|wm|wl).
    """
    n_x = x64.shape[0]
    lhsT = np.zeros((K, n_x), dtype=BF16)
    rhs = np.zeros((K, N), dtype=BF16)
    for d in range(D):
        a = -2.0 * x64[:, d]
        ah, am, al = _split3(a)
        yh, ym, yl = _split3(y64[:, d])
        r = 6 * d
        lhsT[r + 0] = ah
        lhsT[r + 1] = ah
        lhsT[r + 2] = ah
        lhsT[r + 3] = am
        lhsT[r + 4] = am
        lhsT[r + 5] = al
        rhs[r + 0] = yh
        rhs[r + 1] = ym
        rhs[r + 2] = yl
        rhs[r + 3] = yh
        rhs[r + 4] = ym
        rhs[r + 5] = yh
    v = x64[:, 0] ** 2 + x64[:, 1] ** 2
    w = y64[:, 0] ** 2 + y64[:, 1] ** 2
    vh, vm, vl = _split3(v)
    wh, wm, wl = _split3(w)
    one = np.ones((), dtype=BF16)
    lhsT[12], lhsT[13], lhsT[14] = vh, vm, vl
    rhs[12] = one
    rhs[13] = one
    rhs[14] = one
    lhsT[15] = one
    lhsT[16] = one
    lhsT[17] = one
    rhs[15], rhs[16], rhs[17] = wh, wm, wl
    return lhsT, rhs


def kernel(point_set1, point_set2, point_set3):
    from concourse.bass_utils import run_bass_kernel_spmd

    sets64 = [
        np.asarray(point_set1, dtype=np.float64).reshape(B, N, D),
        np.asarray(point_set2, dtype=np.float64).reshape(B, N, D),
        np.asarray(point_set3, dtype=np.float64).reshape(B, N, D),
    ]

    nc = _get_nc()
    in_maps = []
    for c in range(N_CORES):
        lh = np.zeros((UNITS_PER_CORE, K, HALF), dtype=BF16)
        rh = np.zeros((UNITS_PER_CORE, K, N), dtype=BF16)
        for s, (p, b, h) in enumerate(UNITS[c * UNITS_PER_CORE:(c + 1) * UNITS_PER_CORE]):
            xi, yi = PAIRS[p]
            x64 = sets64[xi][b, HALF * h : HALF * (h + 1)]
            y64 = sets64[yi][b]
            lh[s], rh[s] = _prep_unit(x64, y64)
        in_maps.append({"lhst_in": lh, "rhs_in": rh})

    res = run_bass_kernel_spmd(
        nc, in_maps, core_ids=list(range(N_CORES)), trace=False
    )

    # Gather: rowsq per (pair, batch) [N], colsq partial mins.
    rowsq = np.full((3, B, N), np.inf, dtype=np.float64)
    colsq = np.full((3, B, N), np.inf, dtype=np.float64)
    for c in range(N_CORES):
        rmins = np.asarray(res.results[c]["rowmin_out"], dtype=np.float64)
        # [UNITS, XT//4, 128, 8*2*512] bf16 partial col-mins; the free axis
        # is (chunk j, pair, 512) so m = 512*j + f after the pair/part mins
        cmins = res.results[c]["colmin_out"].astype(np.float32)
        cmins = cmins.reshape(UNITS_PER_CORE, XT // 4, 128, 8, 2, 512)
        for s, (p, b, h) in enumerate(UNITS[c * UNITS_PER_CORE:(c + 1) * UNITS_PER_CORE]):
            # rowmins[pidx, i] = min sq for x-point 128*i + pidx (+ half offset)
            rowsq[p, b, HALF * h : HALF * (h + 1)] = (
                rmins[s].T.reshape(-1)
            )
            part = cmins[s].min(axis=(0, 1, 3)).reshape(-1)  # [4096]
            colsq[p, b] = np.minimum(colsq[p, b], part)

    ch = np.empty((3, B), dtype=np.float64)
    for p in range(3):
        for b in range(B):
            d1 = np.sqrt(np.maximum(rowsq[p, b], 0.0)).mean()
            d2 = np.sqrt(np.maximum(colsq[p, b], 0.0)).mean()
            ch[p, b] = 0.5 * (d1 + d2)

    lss = MARGIN - ch * LOSS_WEIGHT          # [3, B]
    out = lss.mean(axis=0)                   # [B]
    return out.astype(np.float32)


# revision 32
# speedup vs baseline: 1.0005x; 1.0005x over previous
"""ChamferLoss2D Trainium2 kernel (8 NeuronCores, SPMD).

Problem: three point sets [4, 4096, 2]; pairwise chamfer losses between
(p1,p2), (p1,p3), (p2,p3); output[b] = MARGIN - mean of the three
chamfer distances.

Algorithm:
  - For each (pair, batch): squared-distance matrix sq[n, m] computed
    entirely on the TensorEngine as a K=18 bf16 matmul using 3-way
    hi/mid/lo bf16 splits of (-2x), y, |x|^2, |y|^2 (error ~2e-7, on
    par with fp32 reference numerics).
  - min(sqrt(sq)) = sqrt(min(sq)): only row/col mins of sq are needed
    on device; sqrt + means happen on host.
  - ScalarE casts PSUM fp32 -> SBUF bf16; VectorE does both min passes
    in bf16 at 2x (tensor_tensor min; row direction via a fold chain
    batched over 4 x-tiles with 3D APs, then one small tensor_reduce).
  - Sharding: 12 (pair, batch) units x 2 halves of the x/query axis =
    24 half-units, 3 per core. Row mins come back fully reduced; col
    mins come back as [128, 4096] partials, min-combined on host.
"""

import numpy as np
import ml_dtypes

BF16 = ml_dtypes.bfloat16

B = 4
N = 4096
D = 2
MARGIN = 1.0
LOSS_WEIGHT = 1.0

N_CORES = 8
HALF = N // 2           # 2048 x-points per half-unit
XT = HALF // 128        # 16 x-tiles per half-unit
UNITS_PER_CORE = 3
K = 18                  # matmul contraction rows

PAIRS = ((0, 1), (0, 2), (1, 2))
# 24 half-units: (pair_idx, batch, half) in fixed order, 3 per core.
UNITS = [(p, b, h) for p in range(3) for b in range(B) for h in range(2)]

_NC_CACHE = {}


def _split3(v64):
    """3-way bf16 split of a float64 array: v ~= h + m + l (residual ~2^-27)."""
    h = v64.astype(BF16)
    r = v64 - h.astype(np.float64)
    m = r.astype(BF16)
    r2 = r - m.astype(np.float64)
    l = r2.astype(BF16)
    return h, m, l


# Engine-completion sems are named "<proc>_<n>". An instruction waiting on
# its OWN engine's completion sem is redundant: all five engines complete
# in program order (PE MMs end pc-monotone; DVE/ACT drain per op), so by
# issue time every earlier own-engine instruction has already bumped the
# sem. DMA-queue sems (DMASW*/DMAHW*) are NOT engine-ordered - keep those.
_ENGINE_SEM_PREFIX = {
    "PE": "PE_",
    "Activation": "Activation_",
    "DVE": "DVE_",
    "Pool": "Pool_",
    "SP": "SP_",
}


def _legalize_sync_waits(nc, sem_by_name):
    """This image's walrus rejects >1 sem-wait on many instruction structs.

    1. Drop redundant own-engine completion waits.
    2. Keep the first remaining wait on the instruction; hoist extras onto
       wait_ge (InstEventSemaphore) carriers inserted immediately before it
       on the same engine (per-engine program order is list order within a
       basic block). Carriers are emitted via the real engine builders (so
       they are well-formed), then relocated."""

    def grab_carrier(engine, sem, value):
        bi = nc.engines[engine].wait_ge(sem, value)
        carrier = bi.ins
        # The builder appended it to the current (tail) bb; remove it.
        cur = nc.cur_bb.bb
        tl = cur.instructions
        assert tl[-1].name == carrier.name, (tl[-1].name, carrier.name)
        cur.instructions = tl[:-1]
        return carrier

    for f in nc.m.functions:
        for bb in f.blocks:
            insts = list(bb.instructions)
            out = []
            changed = False
            for inst in insts:
                si = inst.sync_info
                waits = list(si.on_wait) if si is not None else []
                if len(waits) > 1:
                    pfx = _ENGINE_SEM_PREFIX.get(getattr(inst.engine, "value", ""))
                    if pfx is not None:
                        kept = [w for w in waits if not w.ant_name.startswith(pfx)]
                    else:
                        kept = waits
                    for w in kept[1:]:
                        h = sem_by_name.get(w.ant_name)
                        if h is None:
                            raise RuntimeError(f"unknown sem {w.ant_name}")
                        out.append(grab_carrier(inst.engine, h, w.wait_value))
                    si.on_wait = kept[:1]
                    inst.sync_info = si
                    changed = True
                out.append(inst)
            if changed:
                bb.instructions = out


def _make_patched_tile_context():
    """Tail-drain workaround + global sync-wait legalization."""
    from concourse import tile
    from concourse.vector_clock import ScopedClock

    class PatchedTileContext(tile.TileContext):
        def _drain_and_barrier(self, tick_clock, wait_clock):
            nc = self.nc
            assert self.sems is not None
            sem_by_name = {h.name: h for h in self.sems.allocated().values()}
            _legalize_sync_waits(nc, sem_by_name)
            carrier = nc.sync.nop()
            wait_clock.add_sem_waits(
                carrier.ins, ScopedClock({None: tick_clock.global_clock})
            )
            waits = list(carrier.ins.sync_info.on_wait)
            if waits:
                si = carrier.ins.sync_info
                si.on_wait = []
                carrier.ins.sync_info = si
                for w in waits:
                    h = sem_by_name.get(w.ant_name)
                    if h is None:
                        raise RuntimeError(f"unknown tail sem {w.ant_name}")
                    nc.sync.wait_ge(h, w.wait_value)
            nc.sync.drain()

            # Minimal tail: the SP waits above already gate on all engine /
            # DMA completion sems; skip the expensive EVSEM butterfly
            # (2x all-engine barrier + 27 sem clears, ~10us) that the stock
            # TileContext emits. Each engine's stream simply ends; NEFF
            # completion waits for all engines and DMA queues regardless.
            popped = nc._tile_sem_poison_stack.pop()
            assert popped is self._sem_poison

    return PatchedTileContext


def _build_nc():
    import concourse.bass as bass
    from concourse import mybir

    PatchedTileContext = _make_patched_tile_context()
    dt = mybir.dt
    AluOp = mybir.AluOpType

    nc = bass.Bass(trn_type="TRN2")
    lhst_in = nc.dram_tensor(
        "lhst_in", [UNITS_PER_CORE, K, HALF], dt.bfloat16, kind="ExternalInput"
    )
    rhs_in = nc.dram_tensor(
        "rhs_in", [UNITS_PER_CORE, K, N], dt.bfloat16, kind="ExternalInput"
    )
    # col-direction: ship per-pair-of-x-tiles partials; the remaining min
    # folds run on the host (DVE relief; DMA engines are nearly idle)
    colmin_out = nc.dram_tensor(
        "colmin_out",
        [UNITS_PER_CORE, XT // 4, 128, 2 * N],
        dt.bfloat16,
        kind="ExternalOutput",
    )
    rowmin_out = nc.dram_tensor(
        "rowmin_out", [UNITS_PER_CORE, 128, XT], dt.float32, kind="ExternalOutput"
    )

    with PatchedTileContext(nc) as tc:
        with (
            tc.tile_pool(name="weights", bufs=2) as wpool,
            tc.tile_pool(name="sq", bufs=3) as sqpool,
            tc.tile_pool(name="acc", bufs=2) as accpool,
            tc.tile_pool(name="tmp", bufs=2) as tmppool,
            tc.tile_pool(name="psum", bufs=2, space="PSUM") as pspool,
        ):
            for u in range(UNITS_PER_CORE):
                # operands replicated at partition offsets 0/32/64/96 so four
                # K=18 matmuls can run concurrently in distinct PE row groups
                lhsT = wpool.tile([96 + K, HALF], dt.bfloat16, tag="lhsT")
                rhs = wpool.tile([96 + K, N], dt.bfloat16, tag="rhs")
                for m in range(4):
                    nc.sync.dma_start(lhsT[32 * m : 32 * m + K, :], lhst_in[u])
                    nc.sync.dma_start(rhs[32 * m : 32 * m + K, :], rhs_in[u])

                rowmins = accpool.tile([128, XT], dt.float32, tag="rowmins")

                for g in range(XT // 4):  # groups of 4 x-tiles
                    # chunk-major layout: flat [128, 16384] tile; logical
                    # structure (chunk j, member m, 512) via rearranged views
                    sq4 = sqpool.tile([128, 8 * 4 * 512], dt.bfloat16, tag="sq4")
                    sq4v = sq4[:].rearrange("p (j m f) -> p j m f", j=8, m=4)
                    for j in range(8):  # 512-wide m-chunks
                        ps = pspool.tile([128, 2048], dt.float32, tag="ps")
                        for m in range(4):  # concurrent row-group members
                            i = 4 * g + m
                            nc.tensor.matmul(
                                ps[:, 512 * m : 512 * (m + 1)],
                                lhsT[32 * m : 32 * m + K, 128 * i : 128 * (i + 1)],
                                rhs[32 * m : 32 * m + K, 512 * j : 512 * (j + 1)],
                                tile_position=(32 * m, 0),
                            )
                        # fp32 PSUM -> bf16 SBUF cast on ScalarE: both APs
                        # plain dense 2D (3D/rearranged APs cost ~+390ns/op)
                        nc.scalar.copy(
                            sq4[:, 2048 * j : 2048 * (j + 1)], ps[:]
                        )
                    # col-direction: one batched pair-min (members {0,2} vs
                    # {1,3}) at DVE 2x, then DMA the partial to DRAM
                    pc = tmppool.tile([128, 8, 2, 512], dt.bfloat16, tag="pc")
                    nc.vector.tensor_tensor(
                        pc[:, :, :, :],
                        sq4v[:, :, 0:4:2, :],
                        sq4v[:, :, 1:4:2, :],
                        op=AluOp.min,
                    )
                    nc.sync.dma_start(colmin_out[u, g], pc[:])
                    # row-direction: fold chunks 8->4->2->1, then within-512,
                    # batched across the 4 members
                    rt = tmppool.tile([128, 4, 4, 512], dt.bfloat16, tag="rt")
                    nc.vector.tensor_tensor(
                        rt[:, :, :, :],
                        sq4v[:, 0:4, :, :],
                        sq4v[:, 4:8, :, :],
                        op=AluOp.min,
                    )
                    for clev in (2, 1):
                        nc.vector.tensor_tensor(
                            rt[:, :clev, :, :],
                            rt[:, :clev, :, :],
                            rt[:, clev : 2 * clev, :, :],
                            op=AluOp.min,
                        )
                    for wlev in (256, 128, 64):
                        nc.vector.tensor_tensor(
                            rt[:, 0, :, :wlev],
                            rt[:, 0, :, :wlev],
                            rt[:, 0, :, wlev : 2 * wlev],
                            op=AluOp.min,
                        )
                    nc.vector.tensor_reduce(
                        rowmins[:, 4 * g : 4 * g + 4],
                        rt[:, 0, :, :64],
                        axis=mybir.AxisListType.X,
                        op=AluOp.min,
                    )

                nc.sync.dma_start(rowmin_out[u], rowmins[:])

    return nc


def _get_nc():
    if "nc" not in _NC_CACHE:
        _NC_CACHE["nc"] = _build_nc()
    return _NC_CACHE["nc"]


def _prep_unit(x64, y64):
    """Build lhsT [K, n_x] and rhs [K, N] bf16 planes for one half-unit.

    sq[n, m] = |x_n|^2 + |y_m|^2 - 2 x_n . y_m, via 3-way bf16 splits:
    per dim d: a = -2 x_d, kept products (ah,yh),(ah,ym),(ah,yl),
    (am,yh),(am,ym),(al,yh); plus (vh|vm|vl, 1) and (1, wh|wm|wl).
    """
    n_x = x64.shape[0]
    lhsT = np.zeros((K, n_x), dtype=BF16)
    rhs = np.zeros((K, N), dtype=BF16)
    for d in range(D):
        a = -2.0 * x64[:, d]
        ah, am, al = _split3(a)
        yh, ym, yl = _split3(y64[:, d])
        r = 6 * d
        lhsT[r + 0] = ah
        lhsT[r + 1] = ah
        lhsT[r + 2] = ah
        lhsT[r + 3] = am
        lhsT[r + 4] = am
        lhsT[r + 5] = al
        rhs[r + 0] = yh
        rhs[r + 1] = ym
        rhs[r + 2] = yl
        rhs[r + 3] = yh
        rhs[r + 4] = ym
        rhs[r + 5] = yh
    v = x64[:, 0] ** 2 + x64[:, 1] ** 2
    w = y64[:, 0] ** 2 + y64[:, 1] ** 2
    vh, vm, vl = _split3(v)
    wh, wm, wl = _split3(w)
    one = np.ones((), dtype=BF16)
    lhsT[12], lhsT[13], lhsT[14] = vh, vm, vl
    rhs[12] = one
    rhs[13] = one
    rhs[14] = one
    lhsT[15] = one
    lhsT[16] = one
    lhsT[17] = one
    rhs[15], rhs[16], rhs[17] = wh, wm, wl
    return lhsT, rhs


def kernel(point_set1, point_set2, point_set3):
    from concourse.bass_utils import run_bass_kernel_spmd

    sets64 = [
        np.asarray(point_set1, dtype=np.float64).reshape(B, N, D),
        np.asarray(point_set2, dtype=np.float64).reshape(B, N, D),
        np.asarray(point_set3, dtype=np.float64).reshape(B, N, D),
    ]

    nc = _get_nc()
    in_maps = []
    for c in range(N_CORES):
        lh = np.zeros((UNITS_PER_CORE, K, HALF), dtype=BF16)
        rh = np.zeros((UNITS_PER_CORE, K, N), dtype=BF16)
        for s, (p, b, h) in enumerate(UNITS[c * UNITS_PER_CORE:(c + 1) * UNITS_PER_CORE]):
            xi, yi = PAIRS[p]
            x64 = sets64[xi][b, HALF * h : HALF * (h + 1)]
            y64 = sets64[yi][b]
            lh[s], rh[s] = _prep_unit(x64, y64)
        in_maps.append({"lhst_in": lh, "rhs_in": rh})

    res = run_bass_kernel_spmd(
        nc, in_maps, core_ids=list(range(N_CORES)), trace=False
    )

    # Gather: rowsq per (pair, batch) [N], colsq partial mins.
    rowsq = np.full((3, B, N), np.inf, dtype=np.float64)
    colsq = np.full((3, B, N), np.inf, dtype=np.float64)
    for c in range(N_CORES):
        rmins = np.asarray(res.results[c]["rowmin_out"], dtype=np.float64)
        # [UNITS, XT//4, 128, 8*2*512] bf16 partial col-mins; the free axis
        # is (chunk j, pair, 512) so m = 512*j + f after the pair/part mins
        cmins = res.results[c]["colmin_out"].astype(np.float32)
        cmins = cmins.reshape(UNITS_PER_CORE, XT // 4, 128, 8, 2, 512)
        for s, (p, b, h) in enumerate(UNITS[c * UNITS_PER_CORE:(c + 1) * UNITS_PER_CORE]):
            # rowmins[pidx, i] = min sq for x-point 128*i + pidx (+ half offset)
            rowsq[p, b, HALF * h : HALF * (h + 1)] = (
                rmins[s].T.reshape(-1)
            )
            part = cmins[s].min(axis=(0, 1, 3)).reshape(-1)  # [4096]
            colsq[p, b] = np.minimum(colsq[p, b], part)

    ch = np.empty((3, B), dtype=np.float64)
    for p in range(3):
        for b in range(B):
            d1 = np.sqrt(np.maximum(rowsq[p, b], 0.0)).mean()
            d2 = np.sqrt(np.maximum(colsq[p, b], 0.0)).mean()
            ch[p, b] = 0.5 * (d1 + d2)

    lss = MARGIN - ch * LOSS_WEIGHT          # [3, B]
    out = lss.mean(axis=0)                   # [B]
    return out.astype(np.float32)
